# revision 1
# baseline (speedup 1.0000x reference)
"""Causal self-attention Trainium2 kernel.

Full inputs in, full output out. Internally: 8 NeuronCores, data-parallel on
batch (2) x tensor-parallel on heads (4 groups of 4 heads). Each core computes
its 4 heads' attention for its batch in a transposed layout (head-dim /
key-dim on partitions) and a partial output projection; the host sums the 4
partial projections per batch and adds b_proj.

Per-core device program (all matmuls bf16 with fp32 PSUM accumulation):
  kqv^T = Wpacked.T @ x^T (+bias)       [768, 2048]   (k/q/v rows per head pair)
  per head: S^T = k^T.T-block @ q^T     [128m x 512n] blocks, causal-trimmed
            P^T = exp(S^T + addmask)    (no max subtraction; scores are O(1))
            U^T = [v|1].T-block @ P^T   rows 0-63 = unnormalized sa^T, row 64 = denom
            sa^T = U^T[0:64] * (1/denom broadcast)
  partial out^T = WprojT.T @ sa^T       [1024, 2048] fp32 -> DRAM
"""
import sys, os
sys.path.insert(0, '/opt/trn_rl_repo')
os.environ.setdefault("JAX_PLATFORMS", "")

import numpy as np
import ml_dtypes

import concourse.bass as bass
import concourse.bacc as bacc
import concourse.tile as tile
import concourse.mybir as mybir
from concourse import bass_utils

B, N, D, H, DH = 2, 2048, 1024, 16, 64
G = 4              # heads per core
NCORES = 8
NCH = 512          # n-chunk width
NJ = N // NCH      # 4 n-chunks
NMB = N // 128     # 16 m-blocks
EW = G * 3 * DH    # 768 packed kqv width per core
bf16 = ml_dtypes.bfloat16
f32 = np.float32
AF = mybir.ActivationFunctionType

_cache = {}


def _build_program():
    nc = bacc.Bacc("TRN2", target_bir_lowering=False, debug=False, num_devices=NCORES)

    xt_d = nc.dram_tensor("xt", [D, N], mybir.dt.bfloat16, kind="ExternalInput").ap()
    w_d = nc.dram_tensor("w", [D, EW], mybir.dt.bfloat16, kind="ExternalInput").ap()
    b_d = nc.dram_tensor("bvec", [EW // 128, 128, 1], mybir.dt.float32, kind="ExternalInput").ap()
    wpt_d = nc.dram_tensor("wpt", [2 * 128, D], mybir.dt.bfloat16, kind="ExternalInput").ap()
    mask_d = nc.dram_tensor("masks", [4, 128, NCH], mybir.dt.bfloat16, kind="ExternalInput").ap()
    id_d = nc.dram_tensor("ident", [128, 128], mybir.dt.bfloat16, kind="ExternalInput").ap()
    ones_d = nc.dram_tensor("ones", [128, 64], mybir.dt.float32, kind="ExternalInput").ap()
    out_d = nc.dram_tensor("outt", [D, N], mybir.dt.float32, kind="ExternalOutput").ap()
    dbg = None
    if os.environ.get("KDBG") == "1":
        dbg = {
            "sa": nc.dram_tensor("dbg_sa", [2, 128, N], mybir.dt.bfloat16, kind="ExternalOutput").ap(),
            "kqvT": nc.dram_tensor("dbg_kqvT", [6, 128, N], mybir.dt.bfloat16, kind="ExternalOutput").ap(),
            "vp": nc.dram_tensor("dbg_vp", [4, 128, 16 * 66], mybir.dt.bfloat16, kind="ExternalOutput").ap(),
        }

    with tile.TileContext(nc) as tc:
        _emit(nc, tc, xt_d, w_d, b_d, wpt_d, mask_d, id_d, ones_d, out_d, dbg)

    nc.compile()
    return nc


def _emit(nc, tc, xt_d, w_d, b_d, wpt_d, mask_d, id_d, ones_d, out_d, dbg=None):
    from contextlib import ExitStack

    dt = mybir.dt
    ctx = ExitStack()
    with ctx:
        consts = ctx.enter_context(tc.tile_pool(name="consts", bufs=1))
        work = ctx.enter_context(tc.tile_pool(name="work", bufs=1))

        # ---- constant loads (w/xt interleaved so the first kqv matmuls
        # can start as soon as chunk 0 of each has landed) ----
        xt_sb, w_sb = [], []
        for dc in range(8):
            tw = consts.tile([128, EW], dt.bfloat16, name=f"w{dc}", tag=f"w{dc}")
            nc.sync.dma_start(tw[:], w_d[dc * 128:(dc + 1) * 128, :])
            w_sb.append(tw)
            tx = consts.tile([128, N], dt.bfloat16, name=f"xt{dc}", tag=f"xt{dc}")
            nc.sync.dma_start(tx[:], xt_d[dc * 128:(dc + 1) * 128, :])
            xt_sb.append(tx)
        b_sb = []
        for i in range(EW // 128):
            t = consts.tile([128, 1], dt.float32, name=f"b{i}", tag=f"b{i}")
            nc.sync.dma_start(t[:], b_d[i])
            b_sb.append(t)
        wpt_sb = []
        for kc in range(2):
            t = consts.tile([128, D], dt.bfloat16, name=f"wpt{kc}", tag=f"wpt{kc}")
            nc.sync.dma_start(t[:], wpt_d[kc * 128:(kc + 1) * 128, :])
            wpt_sb.append(t)
        mask_sb = []
        for r in range(4):
            t = consts.tile([128, NCH], dt.bfloat16, name=f"mask{r}", tag=f"mask{r}")
            nc.sync.dma_start(t[:], mask_d[r])
            mask_sb.append(t)
        ident = consts.tile([128, 128], dt.bfloat16, name="ident", tag="ident")
        nc.sync.dma_start(ident[:], id_d[:])
        ones_sb = consts.tile([128, 64], dt.float32, name="ones", tag="ones")
        nc.sync.dma_start(ones_sb[:], ones_d[:])

        # persistent kqv^T, v', sa^T buffers
        kqvT = [work.tile([128, N], dt.bfloat16, name=f"kqvT{i}", tag=f"kqvT{i}")
                for i in range(6)]
        vp = [work.tile([128, NMB, 66], dt.bfloat16, name=f"vp{h}", tag=f"vp{h}")
              for h in range(G)]
        saT = [work.tile([128, N], dt.bfloat16, name=f"saT{kc}", tag=f"saT{kc}")
               for kc in range(2)]

        # per-head slices (pair packing [k_e|k_o|q_e|q_o|v_e|v_o])
        def head_slices(h):
            p, o = h // 2, (h % 2) * 64
            kT = kqvT[3 * p][o:o + 64, :]
            qT = kqvT[3 * p + 1][o:o + 64, :]
            vT = kqvT[3 * p + 2][o:o + 64, :]
            return kT, qT, vT, o

        # ---- fused emission ----
        # PE-bound kqv matmuls are overlapped with the ScalarE-exp-bound
        # attention windows: pair 0's kqv runs up front, pair 1's kqv groups
        # are scattered into pair 0's attention as PE filler; pair 1's
        # v-transposes and the output projection fill pair 1's attention
        # window. This keeps PE dense (HAM clock stays at 2.4GHz) and
        # balances PE vs ScalarE.
        # 8 PSUM banks, all pools open for the whole kernel (no mid-stream
        # pool swaps): ps(3, shared by S tiles / denom-broadcasts /
        # v-transposes) + pu(2) + pk(1) + pp(2). kqv psum groups alternate
        # between pk and pp (pp only carries projections in phase C, after
        # the last kqv group is long done).
        ps = ctx.enter_context(tc.tile_pool(name="ps", bufs=4, space="PSUM"))
        pu = ctx.enter_context(tc.tile_pool(name="pu", bufs=2, space="PSUM"))
        pp = ctx.enter_context(tc.tile_pool(name="pp", bufs=2, space="PSUM"))
        pPool = ctx.enter_context(tc.tile_pool(name="pP", bufs=8))
        paux = ctx.enter_context(tc.tile_pool(name="paux", bufs=2))
        pout = ctx.enter_context(tc.tile_pool(name="pout", bufs=3))

        def emit_kqv_group(mc, jj, bias_on_dve):
            # kqv psum shares the projection pool slots (kqv groups all
            # finish in B0, before the first projection in C)
            ps_t = pp.tile([128, NCH], dt.float32, tag="pp", name="kqvp")
            for dc in range(8):
                nc.tensor.matmul(
                    ps_t[:],
                    w_sb[dc][:, mc * 128:(mc + 1) * 128],
                    xt_sb[dc][:, jj * NCH:(jj + 1) * NCH],
                    start=(dc == 0), stop=(dc == 7),
                )
            dst = kqvT[mc][:, jj * NCH:(jj + 1) * NCH]
            if bias_on_dve:
                nc.vector.tensor_scalar_add(dst, ps_t[:], b_sb[mc][:])
            else:
                nc.scalar.activation(dst, ps_t[:], AF.Identity, bias=b_sb[mc][:])

        def emit_vp_quad(h, q):
            # transpose 4 v-blocks into one psum tile, one strided copy out
            _, _, vT_h, o = head_slices(h)
            tp = ps.tile([128, 256], dt.bfloat16, tag="s", name="vtp")
            for i in range(4):
                mb = 4 * q + i
                nc.tensor.matmul(
                    tp[:, 64 * i:64 * (i + 1)],
                    vT_h[:, mb * 128:(mb + 1) * 128],
                    ident[o:o + 64, o:o + 64],
                    is_transpose=True, skip_group_check=True,
                )
            nc.vector.tensor_copy(vp[h][:, 4 * q:4 * q + 4, 0:64], tp[:])

        def emit_proj_oc(j, oc):
            nsl = slice(j * NCH, (j + 1) * NCH)
            pp_t = pp.tile([128, NCH], dt.float32, tag="pp")
            for kc in range(2):
                nc.tensor.matmul(
                    pp_t[:],
                    wpt_sb[kc][:, oc * 128:(oc + 1) * 128],
                    saT[kc][:, nsl],
                    start=(kc == 0), stop=(kc == 1),
                )
            o_t = pout.tile([128, NCH], dt.float32, tag="o")
            nc.vector.tensor_copy(o_t[:], pp_t[:])
            nc.sync.dma_start(out_d[oc * 128:(oc + 1) * 128, nsl], o_t[:])

        def emit_attn_chunk(j, p, fillers):
            """Attention for chunk j, head pair p, with `fillers` (zero-arg
            emitters of dependency-free PE work) spread across the m-loop."""
            nsl = slice(j * NCH, (j + 1) * NCH)
            nm = 4 * (j + 1)
            pair = (2 * p, 2 * p + 1)
            u_t = {h: pu.tile([65, NCH], dt.float32, tag="u", name=f"u{h}")
                   for h in pair}
            p_tiles = {h: [None] * nm for h in pair}
            offs = [0] * nm
            fill = list(fillers)
            # spread fillers over m-block boundaries (plus a tail flush)
            per_step = max(1, -(-len(fill) // max(nm, 1)))

            def emit_s(h, mi):
                kT, qT, _, _ = head_slices(h)
                r = mi - 4 * j
                off = 128 * r if r > 0 else 0
                offs[mi] = off
                s_t = ps.tile([128, NCH], dt.float32, tag="s")
                nc.tensor.matmul(
                    s_t[:, off:],
                    kT[:, mi * 128:(mi + 1) * 128],
                    qT[:, j * NCH + off:(j + 1) * NCH],
                    start=True, stop=True,
                )
                p_t = pPool.tile([128, NCH], dt.bfloat16, tag="p")
                if r >= 0:
                    e_t = pPool.tile([128, NCH], dt.bfloat16, tag="e")
                    nc.scalar.activation(e_t[:, off:], s_t[:, off:], AF.Exp)
                    nc.vector.tensor_mul(
                        p_t[:, off:], e_t[:, off:], mask_sb[r][:, off:])
                else:
                    nc.scalar.activation(p_t[:, off:], s_t[:, off:], AF.Exp)
                p_tiles[h][mi] = p_t

            def emit_pv(h, mi):
                off = offs[mi]
                nc.tensor.matmul(
                    u_t[h][:, off:],
                    vp[h][:, mi, 0:65],
                    p_tiles[h][mi][:, off:],
                    start=(mi == 0), stop=(mi == nm - 1),
                    skip_group_check=True,
                )

            depth = 2
            for mi in range(nm):
                for n_ in range(per_step):
                    if fill:
                        fill.pop(0)()
                for h in pair:
                    emit_s(h, mi)
                if mi >= depth:
                    for h in pair:
                        emit_pv(h, mi - depth)
            for mi in range(max(nm - depth, 0), nm):
                for h in pair:
                    emit_pv(h, mi)
            while fill:
                fill.pop(0)()

            # normalize both heads: PE-broadcast the raw denominator row,
            # then reciprocal_approx_fast on the [64, n] SBUF broadcast
            # (the only AP shape the custom op handles on HW), then multiply.
            for h in pair:
                dr_t = paux.tile([65, NCH], dt.float32, tag="dr")
                nc.vector.tensor_copy(dr_t[64:65, :], u_t[h][64:65, :])
                bcp = ps.tile([128, NCH], dt.float32, tag="s", name=f"bcp{h}")
                nc.tensor.matmul(bcp[0:64, :], ones_sb[64:65, 0:64],
                                 dr_t[64:65, :], start=True, stop=True)
                bc = paux.tile([64, NCH], dt.float32, tag="bc")
                nc.vector.tensor_copy(bc[:], bcp[0:64, :])
                rc64 = paux.tile([64, NCH], dt.float32, tag="rc64")
                nc.vector.reciprocal_approx_fast(rc64[:], bc[:])
                kc, row = h // 2, (h % 2) * 64
                if row == 0:
                    nc.vector.tensor_mul(saT[kc][0:64, nsl],
                                         u_t[h][0:64, :], rc64[:])
                else:
                    tmp = paux.tile([64, NCH], dt.bfloat16, tag="tmp")
                    nc.vector.tensor_mul(tmp[:], u_t[h][0:64, :], rc64[:])
                    nc.sync.dma_start(saT[kc][64:128, nsl], tmp[:])

        # A0: kqv pair 0 + v' for heads 0/1
        for h in range(G):
            nc.gpsimd.memset(vp[h][:, :, 64:65], 1.0)
        for mc in (0, 1, 2):
            for jj in range(NJ):
                emit_kqv_group(mc, jj, bias_on_dve=False)
        for q in range(NMB // 4):
            emit_vp_quad(0, q)
            emit_vp_quad(1, q)

        # B0: attention pair 0, with pair 1's kqv scattered in; group count
        # per chunk matches each chunk's ScalarE-surplus (later chunks have
        # more exp work for the fillers to hide behind)
        kqv_fill = [(mc, jj) for jj in range(NJ) for mc in (3, 4, 5)]
        per_chunk = (1, 2, 4, 5)
        pos = 0
        for j in range(NJ):
            todo = kqv_fill[pos:pos + per_chunk[j]]
            pos += per_chunk[j]
            fillers = [
                (lambda mc=mc, jj=jj: emit_kqv_group(mc, jj, bias_on_dve=True))
                for mc, jj in todo
            ]
            emit_attn_chunk(j, 0, fillers)

        # C: attention pair 1, with JIT v-transposes and the projection of
        # finished chunks scattered in
        for j in range(NJ):
            fillers = [lambda jq=j: emit_vp_quad(2, jq),
                       lambda jq=j: emit_vp_quad(3, jq)]
            if j >= 1:
                fillers += [(lambda oc=oc, jj=j - 1: emit_proj_oc(jj, oc))
                            for oc in range(8)]
            emit_attn_chunk(j, 1, fillers)
        for oc in range(8):
            emit_proj_oc(NJ - 1, oc)

        if dbg is not None:
            for kc in range(2):
                nc.sync.dma_start(dbg["sa"][kc], saT[kc][:])
            for i in range(6):
                nc.sync.dma_start(dbg["kqvT"][i], kqvT[i][:])
            for h in range(4):
                nc.sync.dma_start(dbg["vp"][h], vp[h].rearrange("p a b -> p (a b)"))


def _host_prep(x, W_kqv, b_kqv, W_proj):
    """Build the 8 per-core input maps."""
    x = np.asarray(x, dtype=f32)
    W_kqv = np.asarray(W_kqv, dtype=f32)
    b_kqv = np.asarray(b_kqv, dtype=f32)
    W_proj = np.asarray(W_proj, dtype=f32)

    masks = np.zeros((4, 128, NCH), dtype=bf16)
    mm = np.arange(128)[:, None]
    nn = np.arange(NCH)[None, :]
    for r in range(4):
        masks[r] = (nn >= mm + 128 * r).astype(bf16)
    ident = np.eye(128, dtype=bf16)

    in_maps = []
    for c in range(NCORES):
        b, g = c // 4, c % 4
        heads = [4 * g + i for i in range(4)]
        # pack per pair: [k_e | k_o | q_e | q_o | v_e | v_o], q scaled by 1/8
        wcols, bcols = [], []
        for p in range(2):
            he, ho = heads[2 * p], heads[2 * p + 1]
            for sec in range(3):  # k, q, v
                scl = 0.125 if sec == 1 else 1.0
                for h in (he, ho):
                    wcols.append(W_kqv[h][:, sec * 64:(sec + 1) * 64] * scl)
                    bcols.append(b_kqv[h][sec * 64:(sec + 1) * 64] * scl)
        wpack = np.concatenate(wcols, axis=1)            # [1024, 768]
        bpack = np.concatenate(bcols).astype(f32)        # [768]
        in_maps.append({
            "xt": np.ascontiguousarray(x[b].T).astype(bf16),
            "w": wpack.astype(bf16),
            "bvec": bpack.reshape(EW // 128, 128, 1),
            "wpt": np.ascontiguousarray(W_proj[:, 256 * g:256 * (g + 1)].T).astype(bf16),
            "masks": masks,
            "ident": ident,
            "ones": np.ones((128, 64), dtype=f32),
        })
    return in_maps


def run(x, W_kqv, b_kqv, W_proj, b_proj, trace=False, trace_cores=None):
    if "nc" not in _cache:
        _cache["nc"] = _build_program()
    nc = _cache["nc"]
    in_maps = _host_prep(x, W_kqv, b_kqv, W_proj)
    res = bass_utils.run_bass_kernel_spmd(
        nc, in_maps, core_ids=list(range(NCORES)),
        trace=trace, trace_cores=trace_cores,
    )
    b_proj = np.asarray(b_proj, dtype=f32)
    out = np.zeros((B, N, D), dtype=f32)
    for b in range(B):
        acc = res.results[4 * b]["outt"].astype(f32).copy()
        for g in range(1, 4):
            acc += res.results[4 * b + g]["outt"]
        out[b] = acc.T + b_proj[None, :]
    return out, res


def kernel(x, W_kqv, b_kqv, W_proj, b_proj):
    out, _ = run(x, W_kqv, b_kqv, W_proj, b_proj, trace=False)
    return out



# revision 3
# speedup vs baseline: 1.0136x; 1.0136x over previous
"""Causal self-attention Trainium2 kernel, v2.

Full inputs in, full output out. 8 cores: data-parallel on batch (2) x
tensor-parallel on heads (4 groups of 4). Per core: 4 heads as 2 pairs,
transposed layout (keys/head-dim on partitions).

v2 changes vs baseline (240.7us):
- x loaded in 512-col chunks so kqv starts at ~4us (was ~17us); PE warm spin
  (ident transposes) releases the HAM clock throttle before real work.
- S for a head pair lands in ONE 2-bank psum tile [128, 1024] -> one exp
  ACT per (pair, key-block): halves ScalarE ACT count (ACT fixed cost is
  ~352cy/instr).
- Diag masking: exp in place then one in-place DVE mul on [128, 2x128]
  strided triangle region only (was full-width mask mul).
- vp blocks stride 128: even head [v(64)|ones|0..], odd head
  [0(32)|ones|0..|v(64)]; PV lhsT = [128,128] windows (FWL on), odd head's
  U lands partition-aligned at rows 64-127 (denom row 32) -> no sbuf-shift
  DMA; one recip [128,512] per pair (custom op works on [128,n] @ part 0).
- Single window order (0,0),(1,0),(0,1),(1,1),(0,2),(1,2),(1,3),(0,3) with
  kqv/vtrans/proj fillers keeping PE dense (HAM stays warm).
"""
import sys, os
sys.path.insert(0, '/opt/trn_rl_repo')
os.environ.setdefault("JAX_PLATFORMS", "")

import numpy as np
import ml_dtypes

import concourse.bass as bass
import concourse.bacc as bacc
import concourse.tile as tile
import concourse.mybir as mybir
from concourse import bass_utils

B, N, D, H, DH = 2, 2048, 1024, 16, 64
G = 4              # heads per core
NCORES = 8
NCH = 512          # n-chunk width
NJ = N // NCH      # 4 chunks
NMB = N // 128     # 16 m-blocks
EW = G * 3 * DH    # 768 packed kqv width per core
bf16 = ml_dtypes.bfloat16
f32 = np.float32
AF = mybir.ActivationFunctionType

_cache = {}


def _build_program():
    nc = bacc.Bacc("TRN2", target_bir_lowering=False, debug=False, num_devices=NCORES)

    dt = mybir.dt
    ident_d = nc.dram_tensor("ident", [128, 128], dt.bfloat16, kind="ExternalInput").ap()
    xtp_d = nc.dram_tensor("xtp", [128, 8, N], dt.bfloat16, kind="ExternalInput").ap()
    wp_d = nc.dram_tensor("wp", [128, 8, EW], dt.bfloat16, kind="ExternalInput").ap()
    b_d = nc.dram_tensor("bvec", [128, 6], dt.float32, kind="ExternalInput").ap()
    wpt_d = nc.dram_tensor("wpt", [128, 2, D], dt.bfloat16, kind="ExternalInput").ap()
    mask_d = nc.dram_tensor("masks", [128, 2, 128], dt.bfloat16, kind="ExternalInput").ap()
    ones_d = nc.dram_tensor("ones", [128, 64], dt.bfloat16, kind="ExternalInput").ap()
    out_d = nc.dram_tensor("outt", [D, N], dt.bfloat16, kind="ExternalOutput").ap()
    dbg = None
    if os.environ.get("KDBG") == "1":
        dbg = {
            "kqvT": nc.dram_tensor("dbg_kqvT", [6, 128, N], dt.bfloat16, kind="ExternalOutput").ap(),
            "sa": nc.dram_tensor("dbg_sa", [2, 128, N], dt.bfloat16, kind="ExternalOutput").ap(),
            "vp": nc.dram_tensor("dbg_vp", [4, 128, 2048], dt.bfloat16, kind="ExternalOutput").ap(),
        }

    with tile.TileContext(nc) as tc:
        _emit(nc, tc, ident_d, xtp_d, wp_d, b_d, wpt_d, mask_d, ones_d, out_d, dbg)

    nc.compile()
    return nc


def _emit(nc, tc, ident_d, xtp_d, wp_d, b_d, wpt_d, mask_d, ones_d, out_d, dbg=None):
    from contextlib import ExitStack

    dt = mybir.dt
    ctx = ExitStack()
    with ctx:
        consts = ctx.enter_context(tc.tile_pool(name="consts", bufs=1))
        work = ctx.enter_context(tc.tile_pool(name="work", bufs=1))

        # ---- DMA loads, ordered for earliest kqv start ----
        # Input loads split across the two HWDGE queues (sync + scalar) so
        # w and x stream in parallel; scalar's queue is clear this early
        # (first ACT is ~14us in).
        ident = consts.tile([128, 128], dt.bfloat16, name="ident", tag="ident")
        nc.sync.dma_start(ident[:], ident_d[:])
        xt_sb = [consts.tile([128, 8, NCH], dt.bfloat16, name=f"xt{jj}", tag=f"xt{jj}")
                 for jj in range(NJ)]
        w_sb = consts.tile([128, 8, EW], dt.bfloat16, name="w_sb", tag="w")
        nc.sync.dma_start(xt_sb[0][:, 0:4, :], xtp_d[:, 0:4, 0:NCH])
        nc.sync.dma_start(w_sb[:, 0:4, :], wp_d[:, 0:4, :])
        nc.sync.dma_start(xt_sb[0][:, 4:8, :], xtp_d[:, 4:8, 0:NCH])
        nc.sync.dma_start(w_sb[:, 4:8, :], wp_d[:, 4:8, :])
        ones_sb = consts.tile([128, 64], dt.bfloat16, name="ones_sb", tag="ones")
        nc.sync.dma_start(ones_sb[:], ones_d[:])
        mask_sb = consts.tile([128, 2, 128], dt.bfloat16, name="mask_sb", tag="mask")
        nc.sync.dma_start(mask_sb[:], mask_d[:])
        b_sb = consts.tile([128, 6], dt.float32, name="b_sb", tag="b")
        nc.sync.dma_start(b_sb[:], b_d[:])
        for jj in range(1, NJ):
            nc.sync.dma_start(xt_sb[jj][:], xtp_d[:, :, jj * NCH:(jj + 1) * NCH])
        wpt_sb = consts.tile([128, 2, D], dt.bfloat16, name="wpt_sb", tag="wpt")
        nc.sync.dma_start(wpt_sb[:], wpt_d[:])

        # ---- persistent tiles ----
        kqvT = [work.tile([128, N], dt.bfloat16, name=f"kqvT{i}", tag=f"kqvT{i}")
                for i in range(6)]
        # vp[h]: [128 keys, 16 blocks x 128]; even: [v(64)|ones|0*63],
        # odd: [0*32|ones|0*31|v(64)]
        vp = [work.tile([128, NMB, 128], dt.bfloat16, name=f"vp{h}", tag=f"vp{h}")
              for h in range(G)]
        saT = [work.tile([128, N], dt.bfloat16, name=f"saT{p}", tag=f"saT{p}")
               for p in range(2)]

        for h in range(G):
            if h % 2 == 0:
                nc.gpsimd.memset(vp[h][:, :, 64:65], 1.0)
                nc.gpsimd.memset(vp[h][:, :, 65:128], 0.0)
            else:
                nc.gpsimd.memset(vp[h][:, :, 0:32], 0.0)
                nc.gpsimd.memset(vp[h][:, :, 32:33], 1.0)
                nc.gpsimd.memset(vp[h][:, :, 33:64], 0.0)

        # ---- pools ----
        # PSUM budget (8 banks): sS 2x[128,1024] (4) + sU 2x[128,512] (2)
        # + sP kqv/proj acc (1) + sX scratch vtrans/bcast (1)
        sS = ctx.enter_context(tc.tile_pool(name="sS", bufs=2, space="PSUM"))
        sU = ctx.enter_context(tc.tile_pool(name="sU", bufs=1, space="PSUM"))
        sP = ctx.enter_context(tc.tile_pool(name="sP", bufs=1, space="PSUM"))
        sX = ctx.enter_context(tc.tile_pool(name="sX", bufs=1, space="PSUM"))
        pP = ctx.enter_context(tc.tile_pool(name="pP", bufs=6))
        pdr = ctx.enter_context(tc.tile_pool(name="pdr", bufs=2))
        prc = ctx.enter_context(tc.tile_pool(name="prc", bufs=2))
        pout = ctx.enter_context(tc.tile_pool(name="pout", bufs=4))

        # ---- PE warm spin: release HAM throttle while DMAs land ----
        def spin_mms(n):
            sp = sX.tile([128, NCH], dt.bfloat16, tag="x", name="spinp")
            for i in range(n):
                sl = slice(128 * (i % 4), 128 * (i % 4 + 1))
                nc.tensor.matmul(sp[:, sl], ident[:], ident[:],
                                 is_transpose=True, skip_group_check=True)

        spin_mms(32)

        # ---- emitters ----
        def emit_kqv_half(p, s, jj, half, state, on_scalar=False):
            g = 3 * p + s
            if half == 0:
                state["a"] = sP.tile([128, NCH], dt.float32, tag="pp", name="kqvp")
            a = state["a"]
            for dc in range(4 * half, 4 * half + 4):
                nc.tensor.matmul(
                    a[:], w_sb[:, dc, g * 128:(g + 1) * 128],
                    xt_sb[jj][:, dc, :],
                    start=(dc == 0), stop=(dc == 7), skip_group_check=True,
                )
            if half == 1:
                dst = kqvT[g][:, jj * NCH:(jj + 1) * NCH]
                if on_scalar:
                    nc.scalar.activation(dst, a[:], AF.Identity,
                                         bias=b_sb[:, g:g + 1])
                else:
                    nc.vector.tensor_scalar_add(dst, a[:], b_sb[:, g:g + 1])

        def emit_vtrans(h, q):
            p, o = h // 2, (h % 2) * 64
            vT = kqvT[3 * p + 2]
            tp = sX.tile([128, 256], dt.bfloat16, tag="x", name="vtp")
            for i in range(4):
                mb = 4 * q + i
                nc.tensor.matmul(
                    tp[:, 64 * i:64 * (i + 1)],
                    vT[o:o + 64, mb * 128:(mb + 1) * 128],
                    ident[o:o + 64, o:o + 64],
                    is_transpose=True, skip_group_check=True,
                )
            dst_lo = 0 if h % 2 == 0 else 64
            nc.vector.tensor_copy(
                vp[h][:, 4 * q:4 * q + 4, dst_lo:dst_lo + 64],
                tp.rearrange("a (b c) -> a b c", b=4),
            )

        def emit_proj(j, oc, copy_on_scalar=False, acc=None):
            jsl = slice(j * NCH, (j + 1) * NCH)
            if acc is None:
                acc = sP.tile([128, NCH], dt.float32, tag="pp", name="projp")
            for kc in range(2):
                nc.tensor.matmul(
                    acc[:],
                    wpt_sb[:, kc, oc * 128:(oc + 1) * 128],
                    saT[kc][:, jsl],
                    start=(kc == 0), stop=(kc == 1), skip_group_check=True,
                )
            o_t = pout.tile([128, NCH], dt.bfloat16, tag="o", name="o_t")
            if copy_on_scalar:
                nc.scalar.activation(o_t[:], acc[:], AF.Identity)
            else:
                nc.vector.tensor_copy(o_t[:], acc[:])
            dq = nc.scalar if copy_on_scalar else nc.sync
            dq.dma_start(out_d[oc * 128:(oc + 1) * 128, jsl], o_t[:])

        def make_norm(p, j, u_e, u_o, dr_on_scalar=False):
            jsl = slice(j * NCH, (j + 1) * NCH)

            def norm():
                # bf16 denom broadcast (fp32 matmul is 4x slower on PE);
                # bf16 rounding of the denominator adds ~0.2% rel err.
                drt = pdr.tile([128, NCH], dt.bfloat16, tag="dr", name=f"dr{p}{j}")
                if dr_on_scalar:
                    nc.scalar.activation(drt[64:65, :], u_e[64:65, :], AF.Identity)
                    nc.scalar.activation(drt[32:33, :], u_o[32:33, :], AF.Identity)
                else:
                    nc.vector.tensor_copy(drt[64:65, :], u_e[64:65, :])
                    nc.vector.tensor_copy(drt[32:33, :], u_o[32:33, :])
                bcp = sX.tile([128, NCH], dt.float32, tag="x", name=f"bcp{p}{j}")
                nc.tensor.matmul(bcp[0:64, :], ones_sb[64:65, 0:64],
                                 drt[64:65, :], start=True, stop=True,
                                 skip_group_check=True)
                nc.tensor.matmul(bcp[64:128, :], ones_sb[32:33, 0:64],
                                 drt[32:33, :], start=True, stop=True,
                                 skip_group_check=True)
                rct = prc.tile([128, NCH], dt.float32, tag="rc", name=f"rc{p}{j}")
                nc.vector.reciprocal_approx_fast(rct[:], bcp[:])
                nc.vector.tensor_mul(saT[p][0:64, jsl], u_e[0:64, :], rct[0:64, :])
                nc.vector.tensor_mul(saT[p][64:128, jsl], u_o[64:128, :], rct[64:128, :])
            return norm

        def emit_attn_window(p, j, fillers, dr_on_scalar=False):
            nm = 4 * (j + 1)
            jsl = slice(j * NCH, (j + 1) * NCH)
            kT, qT = kqvT[3 * p], kqvT[3 * p + 1]
            u_e = sU.tile([128, NCH], dt.float32, tag="ue", name=f"ue{p}{j}")
            u_o = sU.tile([128, NCH], dt.float32, tag="uo", name=f"uo{p}{j}")
            p_tiles = [None] * nm
            offs = [0] * nm
            fill = list(fillers)
            DEPTH = 3

            def emit_s(mi):
                r = mi - 4 * j
                off = 128 * r if r > 0 else 0
                offs[mi] = off
                s_pair = sS.tile([128, 2 * NCH], dt.float32, tag="s",
                                 name=f"s{p}{j}_{mi}")
                nc.tensor.matmul(
                    s_pair[:, off:NCH],
                    kT[0:64, mi * 128:(mi + 1) * 128],
                    qT[0:64, j * NCH + off:(j + 1) * NCH],
                    start=True, stop=True, skip_group_check=True)
                nc.tensor.matmul(
                    s_pair[:, NCH + off:2 * NCH],
                    kT[64:128, mi * 128:(mi + 1) * 128],
                    qT[64:128, j * NCH + off:(j + 1) * NCH],
                    start=True, stop=True, skip_group_check=True)
                p_pair = pP.tile([128, 2 * NCH], dt.bfloat16, tag="p",
                                 name=f"p{p}{j}_{mi}")
                sv = s_pair.rearrange("a (s c) -> a s c", s=2)[:, :, off:NCH]
                pv = p_pair.rearrange("a (s c) -> a s c", s=2)[:, :, off:NCH]
                nc.scalar.activation(pv, sv, AF.Exp)
                if r >= 0:
                    # triangle mask on the idle GPSIMD (SBUF-only op)
                    pm = p_pair.rearrange("a (s c) -> a s c", s=2)[:, :, off:off + 128]
                    nc.gpsimd.tensor_mul(pm, pm, mask_sb[:])
                p_tiles[mi] = p_pair

            def emit_pv(mi):
                off = offs[mi]
                pw = p_tiles[mi]
                nc.tensor.matmul(
                    u_e[:, off:], vp[2 * p][:, mi, :], pw[:, off:NCH],
                    start=(mi == 0), stop=(mi == nm - 1), skip_group_check=True)
                nc.tensor.matmul(
                    u_o[:, off:], vp[2 * p + 1][:, mi, :], pw[:, NCH + off:2 * NCH],
                    start=(mi == 0), stop=(mi == nm - 1), skip_group_check=True)

            # distribute fillers evenly over all nm steps + DEPTH tail slots
            total, slots = len(fill), nm + DEPTH
            popped = 0
            def pop_to(tgt):
                nonlocal popped
                while popped < tgt and fill:
                    fill.pop(0)()
                    popped += 1
            # S first, then fillers (they run on PE while ScalarE does exp),
            # then the trailing PV — fillers must never delay the S MMs that
            # feed the exp stream.
            for mi in range(nm):
                emit_s(mi)
                pop_to(-(-total * (mi + 1)) // slots)
                if mi >= DEPTH:
                    emit_pv(mi - DEPTH)
            for k, mi in enumerate(range(max(nm - DEPTH, 0), nm)):
                pop_to(-(-total * (nm + k + 1)) // slots)
                emit_pv(mi)
            while fill:
                fill.pop(0)()
            return make_norm(p, j, u_e, u_o, dr_on_scalar)

        # ---- warmup: kqv(p0, *, 0) with parallel psum accumulators; MMs
        # interleaved by dc-halves so PE starts as soon as the first half of
        # x/w has landed ----
        acc_k = sS.tile([128, 2 * NCH], dt.float32, tag="s", name="acc_k")
        acc_q = sS.tile([128, 2 * NCH], dt.float32, tag="s", name="acc_q")
        acc_v = sP.tile([128, NCH], dt.float32, tag="pp", name="acc_v")
        w_accs = [acc_k[:, 0:NCH], acc_q[:, 0:NCH], acc_v[:]]
        for dcg in range(2):
            # spins ahead of each dc-half cover any DMA-arrival wait so the
            # HAM activity window never sees an idle PE during warmup
            spin_mms(8)
            for s in range(3):
                for dc in range(4 * dcg, 4 * dcg + 4):
                    nc.tensor.matmul(
                        w_accs[s], w_sb[:, dc, s * 128:(s + 1) * 128],
                        xt_sb[0][:, dc, :],
                        start=(dc == 0), stop=(dc == 7),
                        skip_group_check=True)
        for s in range(3):
            nc.scalar.activation(kqvT[s][:, 0:NCH], w_accs[s], AF.Identity,
                                 bias=b_sb[:, s:s + 1])
        emit_vtrans(0, 0)
        emit_vtrans(1, 0)

        # ---- window loop ----
        def kqv1(p, s, jj, sc=False):
            state = {}
            return [lambda h=half, st=state:
                    emit_kqv_half(p, s, jj, h, st, on_scalar=sc)
                    for half in range(2)]

        def kqv3(p, jj, sc=False):
            out = []
            for s in range(3):
                out += kqv1(p, s, jj, sc)
            return out

        def vt2(p, q):
            return [lambda: emit_vtrans(2 * p, q), lambda: emit_vtrans(2 * p + 1, q)]

        def pr(j, ocs):
            return [(lambda oc=oc, jj=j: emit_proj(jj, oc)) for oc in ocs]

        def spin_burst(n=8):
            def f():
                spin_mms(n)
            return f

        def spins(k, n=8):
            return [spin_burst(n) for _ in range(k)]

        windows = [(0, 0), (1, 0), (0, 1), (1, 1), (0, 2), (1, 2), (1, 3), (0, 3)]
        wfill = [
            kqv3(1, 0, sc=True) + vt2(1, 0),
            kqv3(0, 1, sc=True) + vt2(0, 1),
            kqv3(1, 1, sc=True) + vt2(1, 1) + pr(0, range(0, 4)),
            kqv3(0, 2) + vt2(0, 2) + pr(0, range(4, 8)),
            kqv3(1, 2) + vt2(1, 2) + pr(1, range(0, 4)),
            kqv3(1, 3) + vt2(1, 3) + pr(1, range(4, 8)),
            kqv3(0, 3) + vt2(0, 3) + pr(2, range(0, 8)) + spins(4),
            spins(16),
        ]
        prenorm = None
        for wi, ((p, j), fills) in enumerate(zip(windows, wfill)):
            fillers = ([prenorm] if prenorm else []) + fills
            prenorm = emit_attn_window(p, j, fillers, dr_on_scalar=(wi < 3))
        prenorm()
        # tail: proj(3) double-buffered through the now-free attention psum
        # banks (sS), copies split DVE/ScalarE, spins keep HAM warm
        for oc in range(8):
            tacc = sS.tile([128, 2 * NCH], dt.float32, tag="s",
                           name=f"tacc{oc}")[:, 0:NCH]
            emit_proj(NJ - 1, oc, copy_on_scalar=(oc % 2 == 1), acc=tacc)
            spin_burst(4)()

        if dbg is not None:
            for i in range(6):
                nc.sync.dma_start(dbg["kqvT"][i], kqvT[i][:])
            for p in range(2):
                nc.sync.dma_start(dbg["sa"][p], saT[p][:])
            for h in range(G):
                nc.sync.dma_start(dbg["vp"][h], vp[h].rearrange("p a b -> p (a b)"))


def _host_prep(x, W_kqv, b_kqv, W_proj):
    x = np.asarray(x, dtype=f32)
    W_kqv = np.asarray(W_kqv, dtype=f32)
    b_kqv = np.asarray(b_kqv, dtype=f32)
    W_proj = np.asarray(W_proj, dtype=f32)

    ident = np.eye(128, dtype=bf16)
    tri = (np.arange(128)[None, :] >= np.arange(128)[:, None]).astype(bf16)
    masks = np.stack([tri, tri], axis=1)              # [128, 2, 128]
    ones = np.ones((128, 64), dtype=bf16)

    in_maps = []
    for c in range(NCORES):
        b, g = c // 4, c % 4
        heads = [4 * g + i for i in range(4)]
        wcols, bcols = [], []
        for p in range(2):
            he, ho = heads[2 * p], heads[2 * p + 1]
            for sec in range(3):  # k, q, v
                scl = 0.125 if sec == 1 else 1.0
                cols = []
                bc = []
                for h in (he, ho):
                    cols.append(W_kqv[h][:, sec * 64:(sec + 1) * 64] * scl)
                    bc.append(b_kqv[h][sec * 64:(sec + 1) * 64] * scl)
                wcols.append(np.concatenate(cols, axis=1))
                bcols.append(np.concatenate(bc))
        wpack = np.concatenate(wcols, axis=1)            # [1024, 768]
        bpack = np.stack(bcols, axis=1).astype(f32)      # [128, 6]
        xt = np.ascontiguousarray(x[b].T).astype(bf16)   # [1024, 2048]
        xtp = xt.reshape(8, 128, N).transpose(1, 0, 2)   # [128, 8, 2048]
        wp = wpack.astype(bf16).reshape(8, 128, EW).transpose(1, 0, 2)
        wpt = np.ascontiguousarray(W_proj[:, 256 * g:256 * (g + 1)].T).astype(bf16)
        wptp = wpt.reshape(2, 128, D).transpose(1, 0, 2)  # [128, 2, 1024]
        in_maps.append({
            "ident": ident,
            "xtp": np.ascontiguousarray(xtp),
            "wp": np.ascontiguousarray(wp),
            "bvec": bpack,
            "wpt": np.ascontiguousarray(wptp),
            "masks": np.ascontiguousarray(masks),
            "ones": ones,
        })
    return in_maps


def run(x, W_kqv, b_kqv, W_proj, b_proj, trace=False, trace_cores=None):
    if "nc" not in _cache:
        _cache["nc"] = _build_program()
    nc = _cache["nc"]
    in_maps = _host_prep(x, W_kqv, b_kqv, W_proj)
    res = bass_utils.run_bass_kernel_spmd(
        nc, in_maps, core_ids=list(range(NCORES)),
        trace=trace, trace_cores=trace_cores,
    )
    b_proj = np.asarray(b_proj, dtype=f32)
    out = np.zeros((B, N, D), dtype=f32)
    for b in range(B):
        acc = res.results[4 * b]["outt"].astype(f32).copy()
        for g in range(1, 4):
            acc += res.results[4 * b + g]["outt"]
        out[b] = acc.T + b_proj[None, :]
    return out, res


def kernel(x, W_kqv, b_kqv, W_proj, b_proj):
    out, _ = run(x, W_kqv, b_kqv, W_proj, b_proj, trace=False)
    return out
